# revision 8
# baseline (speedup 1.0000x reference)
"""MoE (top-2 of 8 experts) Trainium2 kernel — fp8 DoubleRow version.

Strategy: expert-parallel across the 8 NeuronCores (host routes tokens,
core e computes expert e's MLP over its gathered tokens). The matmuls run
in fp8(e4m3) DoubleRow mode — one DR instruction contracts TWO 128-row
k-tiles in 0.5 cycles per output column (4x the fp32r rate) — with a
hi/lo split-correction that keeps the end-to-end relative error ~2e-3:

  operand a is stored as a_hi = e4m3(a) and a_lo = e4m3(a - a_hi); the
  product a·w is assembled from three rank-K products
      a_hi·w_hi + a_hi·w_lo + a_lo·w_hi       (a_lo·w_lo ~ 2^-8, dropped)
  The DR pair slots compute two rank-128 products per instruction:
    - "plain"  pairs two k-tiles of (a_hi, w_hi): the main term,
    - "paired" puts (w_hi, w_lo) against (a_lo, a_hi) of ONE k-tile: both
      correction terms in one instruction.
  Stage 1 (contraction H=1024, 8 k-tiles): 4 plain + 8 paired = 6 cyc/col
  Stage 2 (contraction I=1408, 11 k-tiles): 6 plain (one zero-padded) +
      11 paired = 8.5 cyc/col
  vs fp32r's 8 and 11 cyc/col — a 1.31x PE-time reduction, and the fp8
  operands halve the DMA bytes.

Scaling: w1 is host-scaled by SW1=32 (so its lo-part stays in e4m3 normal
range), making psum1 = 32·z. Sigmoid reads psum with scale 1/32; the DVE
multiply gives hv = 32·silu(z) (absmax ~212 < e4m3 max 240), which is
split hi/lo for stage 2. w2 is scaled by SW2=32 and the host pre-divides
the gates by SW1·SW2 so the stage-2 gate-multiply absorbs all scales.

Per-core device pipeline (count = max tokens routed to one expert):
  stage 1, chunk-outer: psum[it] group (full 2KB bank, two 256-col DR
    half-sweeps) -> ACT sigmoid -> DVE mul (hv) -> ACT copy-cast (h_hi)
    -> GpSimd sub (h_lo), writing h into hlh [p, slot(lo,hi,zero), it, C]
  stage 2: psum[ht] group -> DVE gate-mul -> DMA out yT [H, C] fp32.
The host transposes and scatter-adds the two expert contributions.
"""

import numpy as np
import ml_dtypes

import concourse.mybir as mybir
from concourse import bacc
from concourse.tile import TileContext
from concourse.bass_utils import run_bass_kernel_spmd

T, H, I, E = 4096, 1024, 1408, 8
TOPK = 2
P = 128
HK = H // P  # 8
IT = I // P  # 11
N_CORES = 8
F32 = mybir.dt.float32
F8 = mybir.dt.float8e4
E4 = ml_dtypes.float8_e4m3
AF = mybir.ActivationFunctionType
DR = mybir.MatmulPerfMode.DoubleRow
SW1 = 32.0
SW2 = 32.0

# most recently built device program (for test harnesses / cost-model timing)
LAST_NC = None


def _chunks(count):
    """512-wide column chunks (one full PSUM bank each) plus an even tail.
    A big first chunk keeps the PE busy longer than the 0.73us/slice w1
    stream, so the chunk-0 it-sweep is never weight-starved."""
    out = []
    rem = count
    while rem > 0:
        c = min(512, rem)
        out.append(c)
        rem -= c
    return out


def _halves(cs):
    """Split a chunk into DR-sized half-sweeps (moving free dim 2*cols must
    stay <= 512, so <= 256 output columns per DR matmul); halves stay even."""
    if cs <= 256:
        return [(0, cs)]
    h0 = (cs // 2 + 1) // 2 * 2
    return [(0, h0), (h0, cs - h0)]


def build_moe_expert_kernel(count):
    """One-expert MLP over `count` gathered tokens (even)."""
    C = count
    assert count % 2 == 0
    nc = bacc.Bacc("TRN2", target_bir_lowering=False, debug=False, num_devices=N_CORES)

    xlh_d = nc.dram_tensor("xlh", [P, 2 * HK * C], F8, kind="ExternalInput").ap()
    w1_d = nc.dram_tensor("whl1", [P, IT * 2 * HK * P], F8, kind="ExternalInput").ap()
    w2_d = nc.dram_tensor("whl2", [P, HK * IT * 2 * P], F8, kind="ExternalInput").ap()
    g_d = nc.dram_tensor("gates", [1, C], F32, kind="ExternalInput").ap()
    y_d = nc.dram_tensor("yT", [H, C], mybir.dt.bfloat16, kind="ExternalOutput").ap()

    # logical views (slot order: w (hi, lo); x and h (lo, hi[, zero]))
    xlh_v = xlh_d.rearrange("p (s k c) -> p s k c", s=2, k=HK)
    w1_v = w1_d.rearrange("p (i s k j) -> p i s k j", i=IT, s=2, k=HK)
    w2_v = w2_d.rearrange("p (h i s j) -> p h i s j", h=HK, i=IT, s=2)
    y_v = y_d.rearrange("(h p) c -> h p c", p=P)  # [HK, 128, C]

    cks = _chunks(count)
    c_starts = [sum(cks[:j]) for j in range(len(cks))]

    with TileContext(nc) as tc:
        with (
            tc.tile_pool(name="w", bufs=1) as wpool,
            tc.tile_pool(name="hv", bufs=3) as hvpool,
            tc.tile_pool(name="y", bufs=3) as ypool,
            tc.tile_pool(name="ps1", bufs=4, space="PSUM") as ps1p,
            tc.tile_pool(name="ps2", bufs=4, space="PSUM") as ps2p,
        ):
            wt1 = wpool.tile([P, IT, 2, HK, P], F8)
            wt2 = wpool.tile([P, HK, IT, 2, P], F8)
            xt = wpool.tile([P, 2, HK, C], F8)
            hlh = wpool.tile([P, 3, IT, C], F8)
            gb = wpool.tile([P, C], F32)

            # DMA issue order = consumption order. The first psum group's
            # plain matmuls need only (w1 it0 hi hk0:2, x_hi hk0:2), so those
            # slivers go first; then the rest of it0/chunk0, the remaining w1
            # slices (paced by the chunk-0 it-sweep), the other x chunks,
            # gates, and w2 per ht.
            cs0 = cks[0]
            nc.sync.dma_start(wt1[:, 0, 0, 0:2], w1_v[:, 0, 0, 0:2])
            nc.sync.dma_start(xt[:, 1, 0:2, :cs0], xlh_v[:, 1, 0:2, :cs0])
            nc.sync.dma_start(wt1[:, 0, 0, 2:], w1_v[:, 0, 0, 2:])
            nc.sync.dma_start(wt1[:, 0, 1], w1_v[:, 0, 1])
            nc.sync.dma_start(xt[:, 1, 2:, :cs0], xlh_v[:, 1, 2:, :cs0])
            nc.sync.dma_start(xt[:, 0, :, :cs0], xlh_v[:, 0, :, :cs0])
            for it in range(1, IT):
                nc.sync.dma_start(wt1[:, it], w1_v[:, it])
            for c0, cs in zip(c_starts[1:], cks[1:]):
                nc.sync.dma_start(
                    xt[:, :, :, c0 : c0 + cs], xlh_v[:, :, :, c0 : c0 + cs]
                )
            nc.sync.dma_start(gb[:], g_d[0].partition_broadcast(P))
            for ht in range(HK):
                nc.sync.dma_start(wt2[:, ht], w2_v[:, ht])

            # the only zero-slot region stage 2 ever reads (it10 plain term)
            nc.vector.memset(hlh[:, 2, IT - 1, :], 0.0)

            def s1_group(it, c0, cs):
                # plains (both halves) first: they only need the hi slots,
                # which the DMA stream delivers before the lo slots
                ps = ps1p.tile([P, 512], F32, tag="ps1")
                for h0, hcs in _halves(cs):
                    a, b = c0 + h0, c0 + h0 + hcs
                    for hkp in range(0, HK, 2):  # plain: x_hi @ w1_hi
                        nc.tensor.matmul(
                            ps[:, h0 : h0 + hcs],
                            wt1[:, it, 0, hkp : hkp + 2, :],
                            xt[:, 1, hkp : hkp + 2, a:b],
                            start=(h0 == 0 and hkp == 0),
                            stop=False,
                            perf_mode=DR,
                        )
                for h0, hcs in _halves(cs):
                    a, b = c0 + h0, c0 + h0 + hcs
                    for hk in range(HK):  # paired: w_hi*x_lo + w_lo*x_hi
                        nc.tensor.matmul(
                            ps[:, h0 : h0 + hcs],
                            wt1[:, it, :, hk, :],
                            xt[:, :, hk, a:b],
                            start=False,
                            stop=(h0 + hcs == cs and hk == HK - 1),
                            perf_mode=DR,
                        )
                # evacuate: hv = psum * sigmoid(psum/SW1) = SW1*silu(z),
                # then split h into e4m3 hi/lo for stage 2
                sg = hvpool.tile([P, 512], F32, tag="sg")
                nc.scalar.activation(
                    sg[:, :cs], ps[:, :cs], AF.Sigmoid, scale=1.0 / SW1
                )
                hv = hvpool.tile([P, 512], F32, tag="hv")
                nc.vector.tensor_mul(out=hv[:, :cs], in0=ps[:, :cs], in1=sg[:, :cs])
                nc.scalar.activation(hlh[:, 1, it, c0 : c0 + cs], hv[:, :cs], AF.Copy)
                nc.gpsimd.tensor_sub(
                    hlh[:, 0, it, c0 : c0 + cs],
                    hv[:, :cs],
                    hlh[:, 1, it, c0 : c0 + cs],
                )

            def s2_group(ht, c0, cs):
                ps = ps2p.tile([P, 512], F32, tag="ps2")
                for h0, hcs in _halves(cs):
                    a, b = c0 + h0, c0 + h0 + hcs
                    for itp in range(0, IT - 1, 2):  # plain: h_hi @ w2_hi
                        nc.tensor.matmul(
                            ps[:, h0 : h0 + hcs],
                            wt2[:, ht, itp : itp + 2, 0, :],
                            hlh[:, 1, itp : itp + 2, a:b],
                            start=(h0 == 0 and itp == 0),
                            stop=False,
                            perf_mode=DR,
                        )
                    # it10 plain, zero-padded second slot
                    nc.tensor.matmul(
                        ps[:, h0 : h0 + hcs],
                        wt2[:, ht, IT - 1, :, :],
                        hlh[:, 1:3, IT - 1, a:b],
                        start=False,
                        stop=False,
                        perf_mode=DR,
                    )
                    for it in range(IT):  # paired: w2_hi*h_lo + w2_lo*h_hi
                        nc.tensor.matmul(
                            ps[:, h0 : h0 + hcs],
                            wt2[:, ht, it, :, :],
                            hlh[:, 0:2, it, a:b],
                            start=False,
                            stop=(h0 + hcs == cs and it == IT - 1),
                            perf_mode=DR,
                        )
                ys = ypool.tile([P, 512], mybir.dt.bfloat16, tag="ys")
                nc.vector.tensor_mul(
                    out=ys[:, :cs], in0=ps[:, :cs], in1=gb[:, c0 : c0 + cs]
                )
                nc.sync.dma_start(y_v[ht][:, c0 : c0 + cs], ys[:, :cs])

            for c0, cs in zip(c_starts, cks):  # stage 1, chunk-outer
                for it in range(IT):
                    s1_group(it, c0, cs)
            for c0, cs in zip(c_starts, cks):  # stage 2
                for ht in range(HK):
                    s2_group(ht, c0, cs)

    nc.compile()
    global LAST_NC
    LAST_NC = nc
    return nc


def route(router_logits):
    """Host-side router: softmax -> top-2 -> renormalize."""
    logits = np.asarray(router_logits, dtype=np.float32)
    m = logits.max(axis=-1, keepdims=True)
    ex = np.exp(logits - m)
    probs = ex / ex.sum(axis=-1, keepdims=True)
    order = np.argsort(-probs, axis=-1, kind="stable")[:, :TOPK]
    rows = np.arange(logits.shape[0])[:, None]
    topk_p = probs[rows, order]
    topk_p = topk_p / topk_p.sum(axis=-1, keepdims=True)
    return order, topk_p.astype(np.float32)


def _q8(a):
    return np.asarray(a, dtype=np.float32).astype(E4)


def kernel(x, router_logits, w1, w2):
    x = np.ascontiguousarray(np.asarray(x, dtype=np.float32))
    w1 = np.asarray(w1, dtype=np.float32)
    w2 = np.asarray(w2, dtype=np.float32)
    t = x.shape[0]

    top2_idx, top2_gate = route(router_logits)

    expert_tokens = []
    expert_gates = []
    for e in range(E):
        sel = np.nonzero(top2_idx == e)
        expert_tokens.append(sel[0])
        expert_gates.append(top2_gate[sel[0], sel[1]])
    counts = [len(ix) for ix in expert_tokens]
    count = max(2, max(counts) + max(counts) % 2)

    nc = build_moe_expert_kernel(count)

    in_maps = []
    for e in range(E):
        cnt = counts[e]
        xe = x[expert_tokens[e]]  # [cnt, H]
        x_hi = _q8(xe)
        x_lo = _q8(xe - x_hi.astype(np.float32))
        xlh = np.zeros((P, 2, HK, count), dtype=E4)
        xlh[:, 0, :, :cnt] = x_lo.reshape(cnt, HK, P).transpose(2, 1, 0)
        xlh[:, 1, :, :cnt] = x_hi.reshape(cnt, HK, P).transpose(2, 1, 0)

        W1 = SW1 * w1[e]  # [I, H]
        W1_hi = _q8(W1)
        W1_lo = _q8(W1 - W1_hi.astype(np.float32))
        # whl1[p, it, slot, hk, j] = W1_s[it*128+j, hk*128+p]
        w1hi_t = W1_hi.reshape(IT, P, HK, P).transpose(3, 0, 2, 1)
        w1lo_t = W1_lo.reshape(IT, P, HK, P).transpose(3, 0, 2, 1)
        whl1 = np.stack([w1hi_t, w1lo_t], axis=2)  # [p, it, 2, hk, j]

        W2 = SW2 * w2[e]  # [H, I]
        W2_hi = _q8(W2)
        W2_lo = _q8(W2 - W2_hi.astype(np.float32))
        # whl2[p, ht, it, slot, j] = W2_s[ht*128+j, it*128+p]
        w2hi_t = W2_hi.reshape(HK, P, IT, P).transpose(3, 0, 2, 1)
        w2lo_t = W2_lo.reshape(HK, P, IT, P).transpose(3, 0, 2, 1)
        whl2 = np.stack([w2hi_t, w2lo_t], axis=3)  # [p, ht, it, 2, j]

        g = np.zeros((1, count), dtype=np.float32)
        g[0, :cnt] = expert_gates[e] / (SW1 * SW2)

        in_maps.append(
            {
                "xlh": np.ascontiguousarray(xlh).reshape(P, -1),
                "whl1": np.ascontiguousarray(whl1).reshape(P, -1),
                "whl2": np.ascontiguousarray(whl2).reshape(P, -1),
                "gates": g,
            }
        )

    res = run_bass_kernel_spmd(nc, in_maps, core_ids=list(range(N_CORES)))
    if not all(np.isfinite(r["yT"]).all() for r in res.results):
        # one retry in case of a transient device fault
        res = run_bass_kernel_spmd(nc, in_maps, core_ids=list(range(N_CORES)))

    out = np.zeros((t, H), dtype=np.float32)
    for e in range(E):
        cnt = counts[e]
        out[expert_tokens[e]] += res.results[e]["yT"][:, :cnt].T.astype(np.float32)
    return out


# revision 12
# speedup vs baseline: 1.0236x; 1.0236x over previous
"""MoE (top-2 of 8 experts) Trainium2 kernel — fp8 DoubleRow version.

Strategy: expert-parallel across the 8 NeuronCores (host routes tokens,
core e computes expert e's MLP over its gathered tokens). The matmuls run
in fp8(e4m3) DoubleRow mode — one DR instruction contracts TWO 128-row
k-tiles in 0.5 cycles per output column (4x the fp32r rate) — with a
hi/lo split-correction that keeps the end-to-end relative error ~2e-3:

  operand a is stored as a_hi = e4m3(a) and a_lo = e4m3(a - a_hi); the
  product a·w is assembled from three rank-K products
      a_hi·w_hi + a_hi·w_lo + a_lo·w_hi       (a_lo·w_lo ~ 2^-8, dropped)
  The DR pair slots compute two rank-128 products per instruction:
    - "plain"  pairs two k-tiles of (a_hi, w_hi): the main term,
    - "paired" puts (w_hi, w_lo) against (a_lo, a_hi) of ONE k-tile: both
      correction terms in one instruction.
  Stage 1 (contraction H=1024, 8 k-tiles): 4 plain + 8 paired = 6 cyc/col
  Stage 2 (contraction I=1408, 11 k-tiles): 6 plain (one zero-padded) +
      11 paired = 8.5 cyc/col
  vs fp32r's 8 and 11 cyc/col — a 1.31x PE-time reduction, and the fp8
  operands halve the DMA bytes.

Scaling: w1 is host-scaled by SW1=32 (so its lo-part stays in e4m3 normal
range), making psum1 = 32·z. Sigmoid reads psum with scale 1/32; the DVE
multiply gives hv = 32·silu(z) (absmax ~212 < e4m3 max 240), which is
split hi/lo for stage 2. w2 is scaled by SW2=32 and the host pre-divides
the gates by SW1·SW2 so the stage-2 gate-multiply absorbs all scales.

Per-core device pipeline (count = max tokens routed to one expert):
  stage 1, chunk-outer: psum[it] group (full 2KB bank, two 256-col DR
    half-sweeps) -> ACT sigmoid -> DVE mul (hv) -> ACT copy-cast (h_hi)
    -> GpSimd sub (h_lo), writing h into hlh [p, slot(lo,hi,zero), it, C]
  stage 2: psum[ht] group -> DVE gate-mul -> DMA out yT [H, C] fp32.
The host transposes and scatter-adds the two expert contributions.
"""

import numpy as np
import ml_dtypes

import concourse.mybir as mybir
from concourse import bacc
from concourse.tile import TileContext
from concourse.bass_utils import run_bass_kernel_spmd

T, H, I, E = 4096, 1024, 1408, 8
TOPK = 2
P = 128
HK = H // P  # 8
IT = I // P  # 11
N_CORES = 8
F32 = mybir.dt.float32
F8 = mybir.dt.float8e4
E4 = ml_dtypes.float8_e4m3
AF = mybir.ActivationFunctionType
DR = mybir.MatmulPerfMode.DoubleRow
SW1 = 32.0
SW2 = 32.0

# most recently built device program (for test harnesses / cost-model timing)
LAST_NC = None


def _chunks(count):
    """512-wide column chunks (one full PSUM bank each) plus an even tail.
    A big first chunk keeps the PE busy longer than the 0.73us/slice w1
    stream, so the chunk-0 it-sweep is never weight-starved."""
    out = []
    rem = count
    while rem > 0:
        c = min(512, rem)
        out.append(c)
        rem -= c
    return out


def _halves(cs):
    """Split a chunk into DR-sized half-sweeps (moving free dim 2*cols must
    stay <= 512, so <= 256 output columns per DR matmul); halves stay even."""
    if cs <= 256:
        return [(0, cs)]
    h0 = (cs // 2 + 1) // 2 * 2
    return [(0, h0), (h0, cs - h0)]


def build_moe_expert_kernel(count):
    """One-expert MLP over `count` gathered tokens (even)."""
    C = count
    assert count % 2 == 0
    nc = bacc.Bacc("TRN2", target_bir_lowering=False, debug=False, num_devices=N_CORES)

    xlh_d = nc.dram_tensor("xlh", [P, 2 * HK * C], F8, kind="ExternalInput").ap()
    w1_d = nc.dram_tensor("whl1", [P, IT * 2 * HK * P], F8, kind="ExternalInput").ap()
    w2_d = nc.dram_tensor("whl2", [P, HK * IT * 2 * P], F8, kind="ExternalInput").ap()
    g_d = nc.dram_tensor("gates", [1, C], F32, kind="ExternalInput").ap()
    y_d = nc.dram_tensor("yT", [H, C], mybir.dt.bfloat16, kind="ExternalOutput").ap()

    # logical views (slot order: w (hi, lo); x and h (lo, hi[, zero]))
    xlh_v = xlh_d.rearrange("p (s k c) -> p s k c", s=2, k=HK)
    w1_v = w1_d.rearrange("p (i s k j) -> p i s k j", i=IT, s=2, k=HK)
    w2_v = w2_d.rearrange("p (h i s j) -> p h i s j", h=HK, i=IT, s=2)
    y_v = y_d.rearrange("(h p) c -> h p c", p=P)  # [HK, 128, C]

    cks = _chunks(count)
    c_starts = [sum(cks[:j]) for j in range(len(cks))]

    with TileContext(nc) as tc:
        with (
            tc.tile_pool(name="w", bufs=1) as wpool,
            tc.tile_pool(name="hv", bufs=3) as hvpool,
            tc.tile_pool(name="y", bufs=6) as ypool,
            tc.tile_pool(name="ps1", bufs=4, space="PSUM") as ps1p,
            tc.tile_pool(name="ps2", bufs=4, space="PSUM") as ps2p,
        ):
            wt1 = wpool.tile([P, IT, 2, HK, P], F8)
            wt2 = wpool.tile([P, HK, IT, 2, P], F8)
            xt = wpool.tile([P, 2, HK, C], F8)
            hlh = wpool.tile([P, 3, IT, C], F8)
            gb = wpool.tile([P, C], F32)

            # DMA issue order = consumption order. The first psum group's
            # plain matmuls need only (w1 it0 hi hk0:2, x_hi hk0:2), so those
            # slivers go first; then the rest of it0/chunk0, the remaining w1
            # slices (paced by the chunk-0 it-sweep), the other x chunks,
            # gates, and w2 per ht.
            cs0 = cks[0]
            nc.sync.dma_start(wt1[:, 0, 0, 0:2], w1_v[:, 0, 0, 0:2])
            nc.sync.dma_start(xt[:, 1, 0:2, :cs0], xlh_v[:, 1, 0:2, :cs0])
            nc.sync.dma_start(wt1[:, 0, 0, 2:], w1_v[:, 0, 0, 2:])
            nc.sync.dma_start(xt[:, 1, 2:, :cs0], xlh_v[:, 1, 2:, :cs0])
            nc.sync.dma_start(wt1[:, 0, 1], w1_v[:, 0, 1])
            nc.sync.dma_start(xt[:, 0, :, :cs0], xlh_v[:, 0, :, :cs0])
            for it in range(1, IT):
                nc.sync.dma_start(wt1[:, it], w1_v[:, it])
            for c0, cs in zip(c_starts[1:], cks[1:]):
                nc.sync.dma_start(
                    xt[:, :, :, c0 : c0 + cs], xlh_v[:, :, :, c0 : c0 + cs]
                )
            nc.sync.dma_start(gb[:], g_d[0].partition_broadcast(P))
            for ht in range(HK):
                nc.sync.dma_start(wt2[:, ht], w2_v[:, ht])

            # the only zero-slot region stage 2 ever reads (it10 plain term)
            nc.vector.memset(hlh[:, 2, IT - 1, :], 0.0)

            def s1_plains(it, c0, cs):
                # plains (both halves) first: they only need the hi slots,
                # which the DMA stream delivers before the lo slots
                ps = ps1p.tile([P, 512], F32, tag="ps1")
                for h0, hcs in _halves(cs):
                    a, b = c0 + h0, c0 + h0 + hcs
                    for hkp in range(0, HK, 2):  # plain: x_hi @ w1_hi
                        nc.tensor.matmul(
                            ps[:, h0 : h0 + hcs],
                            wt1[:, it, 0, hkp : hkp + 2, :],
                            xt[:, 1, hkp : hkp + 2, a:b],
                            start=(h0 == 0 and hkp == 0),
                            stop=False,
                            perf_mode=DR,
                        )
                return ps

            def s1_rest(it, c0, cs, ps):
                for h0, hcs in _halves(cs):
                    a, b = c0 + h0, c0 + h0 + hcs
                    for hk in range(HK):  # paired: w_hi*x_lo + w_lo*x_hi
                        nc.tensor.matmul(
                            ps[:, h0 : h0 + hcs],
                            wt1[:, it, :, hk, :],
                            xt[:, :, hk, a:b],
                            start=False,
                            stop=(h0 + hcs == cs and hk == HK - 1),
                            perf_mode=DR,
                        )
                # evacuate: hv = psum * sigmoid(psum/SW1) = SW1*silu(z),
                # then split h into e4m3 hi/lo for stage 2
                sg = hvpool.tile([P, 512], F32, tag="sg")
                nc.scalar.activation(
                    sg[:, :cs], ps[:, :cs], AF.Sigmoid, scale=1.0 / SW1
                )
                hv = hvpool.tile([P, 512], F32, tag="hv")
                nc.vector.tensor_mul(out=hv[:, :cs], in0=ps[:, :cs], in1=sg[:, :cs])
                nc.scalar.activation(hlh[:, 1, it, c0 : c0 + cs], hv[:, :cs], AF.Copy)
                nc.gpsimd.tensor_sub(
                    hlh[:, 0, it, c0 : c0 + cs],
                    hv[:, :cs],
                    hlh[:, 1, it, c0 : c0 + cs],
                )

            def s2_group(ht, c0, cs):
                ps = ps2p.tile([P, 512], F32, tag="ps2")
                for h0, hcs in _halves(cs):
                    a, b = c0 + h0, c0 + h0 + hcs
                    for itp in range(0, IT - 1, 2):  # plain: h_hi @ w2_hi
                        nc.tensor.matmul(
                            ps[:, h0 : h0 + hcs],
                            wt2[:, ht, itp : itp + 2, 0, :],
                            hlh[:, 1, itp : itp + 2, a:b],
                            start=(h0 == 0 and itp == 0),
                            stop=False,
                            perf_mode=DR,
                        )
                    # it10 plain, zero-padded second slot
                    nc.tensor.matmul(
                        ps[:, h0 : h0 + hcs],
                        wt2[:, ht, IT - 1, :, :],
                        hlh[:, 1:3, IT - 1, a:b],
                        start=False,
                        stop=False,
                        perf_mode=DR,
                    )
                    for it in range(IT):  # paired: w2_hi*h_lo + w2_lo*h_hi
                        nc.tensor.matmul(
                            ps[:, h0 : h0 + hcs],
                            wt2[:, ht, it, :, :],
                            hlh[:, 0:2, it, a:b],
                            start=False,
                            stop=(h0 + hcs == cs and it == IT - 1),
                            perf_mode=DR,
                        )
                ys = ypool.tile([P, 512], mybir.dt.bfloat16, tag="ys")
                nc.vector.tensor_mul(
                    out=ys[:, :cs], in0=ps[:, :cs], in1=gb[:, c0 : c0 + cs]
                )
                nc.sync.dma_start(y_v[ht][:, c0 : c0 + cs], ys[:, :cs])

            cl = list(zip(c_starts, cks))
            # chunk 0: staggered — run DEPTH groups' plains ahead so the PE
            # has hi-slot work while the lo slots / later w1 slices stream in
            DEPTH = 4
            c0_, cs_ = cl[0]
            pss = {}
            for it in range(min(DEPTH, IT)):
                pss[it] = s1_plains(it, c0_, cs_)
            for it in range(IT):
                s1_rest(it, c0_, cs_, pss.pop(it))
                if it + DEPTH < IT:
                    pss[it + DEPTH] = s1_plains(it + DEPTH, c0_, cs_)
            # software pipeline: stage-2 of chunk i runs after stage-1 of
            # chunk i+1, so the PE stays fed while evac chains drain and the
            # small tail chunk lands last (short epilogue)
            for ci in range(1, len(cl)):
                c0_, cs_ = cl[ci]
                for it in range(IT):
                    s1_rest(it, c0_, cs_, s1_plains(it, c0_, cs_))
                pc0, pcs = cl[ci - 1]
                for ht in range(HK):
                    s2_group(ht, pc0, pcs)
            lc0, lcs = cl[-1]
            for ht in range(HK):
                s2_group(ht, lc0, lcs)

    nc.compile()
    global LAST_NC
    LAST_NC = nc
    return nc


def route(router_logits):
    """Host-side router: softmax -> top-2 -> renormalize."""
    logits = np.asarray(router_logits, dtype=np.float32)
    m = logits.max(axis=-1, keepdims=True)
    ex = np.exp(logits - m)
    probs = ex / ex.sum(axis=-1, keepdims=True)
    order = np.argsort(-probs, axis=-1, kind="stable")[:, :TOPK]
    rows = np.arange(logits.shape[0])[:, None]
    topk_p = probs[rows, order]
    topk_p = topk_p / topk_p.sum(axis=-1, keepdims=True)
    return order, topk_p.astype(np.float32)


def _q8(a):
    return np.asarray(a, dtype=np.float32).astype(E4)


def kernel(x, router_logits, w1, w2):
    x = np.ascontiguousarray(np.asarray(x, dtype=np.float32))
    w1 = np.asarray(w1, dtype=np.float32)
    w2 = np.asarray(w2, dtype=np.float32)
    t = x.shape[0]

    top2_idx, top2_gate = route(router_logits)

    expert_tokens = []
    expert_gates = []
    for e in range(E):
        sel = np.nonzero(top2_idx == e)
        expert_tokens.append(sel[0])
        expert_gates.append(top2_gate[sel[0], sel[1]])
    counts = [len(ix) for ix in expert_tokens]
    count = max(2, max(counts) + max(counts) % 2)

    nc = build_moe_expert_kernel(count)

    in_maps = []
    for e in range(E):
        cnt = counts[e]
        xe = x[expert_tokens[e]]  # [cnt, H]
        x_hi = _q8(xe)
        x_lo = _q8(xe - x_hi.astype(np.float32))
        xlh = np.zeros((P, 2, HK, count), dtype=E4)
        xlh[:, 0, :, :cnt] = x_lo.reshape(cnt, HK, P).transpose(2, 1, 0)
        xlh[:, 1, :, :cnt] = x_hi.reshape(cnt, HK, P).transpose(2, 1, 0)

        W1 = SW1 * w1[e]  # [I, H]
        W1_hi = _q8(W1)
        W1_lo = _q8(W1 - W1_hi.astype(np.float32))
        # whl1[p, it, slot, hk, j] = W1_s[it*128+j, hk*128+p]
        w1hi_t = W1_hi.reshape(IT, P, HK, P).transpose(3, 0, 2, 1)
        w1lo_t = W1_lo.reshape(IT, P, HK, P).transpose(3, 0, 2, 1)
        whl1 = np.stack([w1hi_t, w1lo_t], axis=2)  # [p, it, 2, hk, j]

        W2 = SW2 * w2[e]  # [H, I]
        W2_hi = _q8(W2)
        W2_lo = _q8(W2 - W2_hi.astype(np.float32))
        # whl2[p, ht, it, slot, j] = W2_s[ht*128+j, it*128+p]
        w2hi_t = W2_hi.reshape(HK, P, IT, P).transpose(3, 0, 2, 1)
        w2lo_t = W2_lo.reshape(HK, P, IT, P).transpose(3, 0, 2, 1)
        whl2 = np.stack([w2hi_t, w2lo_t], axis=3)  # [p, ht, it, 2, j]

        g = np.zeros((1, count), dtype=np.float32)
        g[0, :cnt] = expert_gates[e] / (SW1 * SW2)

        in_maps.append(
            {
                "xlh": np.ascontiguousarray(xlh).reshape(P, -1),
                "whl1": np.ascontiguousarray(whl1).reshape(P, -1),
                "whl2": np.ascontiguousarray(whl2).reshape(P, -1),
                "gates": g,
            }
        )

    res = run_bass_kernel_spmd(nc, in_maps, core_ids=list(range(N_CORES)))
    if not all(np.isfinite(r["yT"]).all() for r in res.results):
        # one retry in case of a transient device fault
        res = run_bass_kernel_spmd(nc, in_maps, core_ids=list(range(N_CORES)))

    out = np.zeros((t, H), dtype=np.float32)
    for e in range(E):
        cnt = counts[e]
        out[expert_tokens[e]] += res.results[e]["yT"][:, :cnt].T.astype(np.float32)
    return out


# revision 13
# speedup vs baseline: 1.0513x; 1.0270x over previous
"""MoE (top-2 of 8 experts) Trainium2 kernel — fp8 DoubleRow version.

Strategy: expert-parallel across the 8 NeuronCores (host routes tokens,
core e computes expert e's MLP over its gathered tokens). The matmuls run
in fp8(e4m3) DoubleRow mode — one DR instruction contracts TWO 128-row
k-tiles in 0.5 cycles per output column (4x the fp32r rate) — with a
hi/lo split-correction that keeps the end-to-end relative error ~2e-3:

  operand a is stored as a_hi = e4m3(a) and a_lo = e4m3(a - a_hi); the
  product a·w is assembled from three rank-K products
      a_hi·w_hi + a_hi·w_lo + a_lo·w_hi       (a_lo·w_lo ~ 2^-8, dropped)
  The DR pair slots compute two rank-128 products per instruction:
    - "plain"  pairs two k-tiles of (a_hi, w_hi): the main term,
    - "paired" puts (w_hi, w_lo) against (a_lo, a_hi) of ONE k-tile: both
      correction terms in one instruction.
  Stage 1 (contraction H=1024, 8 k-tiles): 4 plain + 8 paired = 6 cyc/col
  Stage 2 (contraction I=1408, 11 k-tiles): 6 plain (one zero-padded) +
      11 paired = 8.5 cyc/col
  vs fp32r's 8 and 11 cyc/col — a 1.31x PE-time reduction, and the fp8
  operands halve the DMA bytes.

Scaling: w1 is host-scaled by SW1=32 (so its lo-part stays in e4m3 normal
range), making psum1 = 32·z. Sigmoid reads psum with scale 1/32; the DVE
multiply gives hv = 32·silu(z) (absmax ~212 < e4m3 max 240), which is
split hi/lo for stage 2. w2 is scaled by SW2=32 and the host pre-divides
the gates by SW1·SW2 so the stage-2 gate-multiply absorbs all scales.

Per-core device pipeline (count = max tokens routed to one expert):
  stage 1, chunk-outer: psum[it] group (full 2KB bank, two 256-col DR
    half-sweeps) -> ACT sigmoid -> DVE mul (hv) -> ACT copy-cast (h_hi)
    -> GpSimd sub (h_lo), writing h into hlh [p, slot(lo,hi,zero), it, C]
  stage 2: psum[ht] group -> DVE gate-mul -> DMA out yT [H, C] fp32.
The host transposes and scatter-adds the two expert contributions.
"""

import numpy as np
import ml_dtypes

import concourse.mybir as mybir
from concourse import bacc
from concourse.tile import TileContext
from concourse.bass_utils import run_bass_kernel_spmd

T, H, I, E = 4096, 1024, 1408, 8
TOPK = 2
P = 128
HK = H // P  # 8
IT = I // P  # 11
N_CORES = 8
F32 = mybir.dt.float32
F8 = mybir.dt.float8e4
E4 = ml_dtypes.float8_e4m3
AF = mybir.ActivationFunctionType
DR = mybir.MatmulPerfMode.DoubleRow
SW1 = 32.0
SW2 = 32.0

# most recently built device program (for test harnesses / cost-model timing)
LAST_NC = None


def _chunks(count):
    """512-wide column chunks (one full PSUM bank each) plus an even tail.
    A big first chunk keeps the PE busy longer than the 0.73us/slice w1
    stream, so the chunk-0 it-sweep is never weight-starved."""
    out = []
    rem = count
    first = min(256, rem)
    out.append(first)
    rem -= first
    while rem > 0:
        c = min(512, rem)
        out.append(c)
        rem -= c
    return out


def _halves(cs):
    """Split a chunk into DR-sized half-sweeps (moving free dim 2*cols must
    stay <= 512, so <= 256 output columns per DR matmul); halves stay even."""
    if cs <= 256:
        return [(0, cs)]
    h0 = (cs // 2 + 1) // 2 * 2
    return [(0, h0), (h0, cs - h0)]


def build_moe_expert_kernel(count):
    """One-expert MLP over `count` gathered tokens (even)."""
    C = count
    assert count % 2 == 0
    nc = bacc.Bacc("TRN2", target_bir_lowering=False, debug=False, num_devices=N_CORES)

    xlh_d = nc.dram_tensor("xlh", [P, 2 * HK * C], F8, kind="ExternalInput").ap()
    w1_d = nc.dram_tensor("whl1", [P, IT * 2 * HK * P], F8, kind="ExternalInput").ap()
    w2_d = nc.dram_tensor("whl2", [P, HK * IT * 2 * P], F8, kind="ExternalInput").ap()
    g_d = nc.dram_tensor("gates", [1, C], F32, kind="ExternalInput").ap()
    y_d = nc.dram_tensor("yT", [H, C], mybir.dt.bfloat16, kind="ExternalOutput").ap()

    # logical views (slot order: w (hi, lo); x and h (lo, hi[, zero]))
    xlh_v = xlh_d.rearrange("p (s k c) -> p s k c", s=2, k=HK)
    w1_v = w1_d.rearrange("p (i s k j) -> p i s k j", i=IT, s=2, k=HK)
    w2_v = w2_d.rearrange("p (h i s j) -> p h i s j", h=HK, i=IT, s=2)
    y_v = y_d.rearrange("(h p) c -> h p c", p=P)  # [HK, 128, C]

    cks = _chunks(count)
    c_starts = [sum(cks[:j]) for j in range(len(cks))]

    with TileContext(nc) as tc:
        with (
            tc.tile_pool(name="w", bufs=1) as wpool,
            tc.tile_pool(name="hv", bufs=3) as hvpool,
            tc.tile_pool(name="y", bufs=6) as ypool,
            tc.tile_pool(name="ps1", bufs=4, space="PSUM") as ps1p,
            tc.tile_pool(name="ps2", bufs=4, space="PSUM") as ps2p,
        ):
            wt1 = wpool.tile([P, IT, 2, HK, P], F8)
            wt2 = wpool.tile([P, HK, IT, 2, P], F8)
            xt = wpool.tile([P, 2, HK, C], F8)
            hlh = wpool.tile([P, 3, IT, C], F8)
            gb = wpool.tile([P, C], F32)

            # DMA issue order = consumption order. The first psum group's
            # plain matmuls need only (w1 it0 hi hk0:2, x_hi hk0:2), so those
            # slivers go first; then the rest of it0/chunk0, the remaining w1
            # slices (paced by the chunk-0 it-sweep), the other x chunks,
            # gates, and w2 per ht.
            cs0 = cks[0]
            nc.sync.dma_start(wt1[:, 0, 0, 0:2], w1_v[:, 0, 0, 0:2])
            nc.sync.dma_start(xt[:, 1, 0:2, :cs0], xlh_v[:, 1, 0:2, :cs0])
            nc.sync.dma_start(wt1[:, 0, 0, 2:], w1_v[:, 0, 0, 2:])
            nc.sync.dma_start(xt[:, 1, 2:, :cs0], xlh_v[:, 1, 2:, :cs0])
            nc.sync.dma_start(wt1[:, 0, 1], w1_v[:, 0, 1])
            nc.sync.dma_start(xt[:, 0, :, :cs0], xlh_v[:, 0, :, :cs0])
            for it in range(1, IT):
                nc.sync.dma_start(wt1[:, it], w1_v[:, it])
            for c0, cs in zip(c_starts[1:], cks[1:]):
                nc.sync.dma_start(
                    xt[:, :, :, c0 : c0 + cs], xlh_v[:, :, :, c0 : c0 + cs]
                )
            nc.sync.dma_start(gb[:], g_d[0].partition_broadcast(P))
            for ht in range(HK):
                nc.sync.dma_start(wt2[:, ht], w2_v[:, ht])

            # the only zero-slot region stage 2 ever reads (it10 plain term)
            nc.vector.memset(hlh[:, 2, IT - 1, :], 0.0)

            def s1_plains(it, c0, cs):
                # plains (both halves) first: they only need the hi slots,
                # which the DMA stream delivers before the lo slots
                ps = ps1p.tile([P, 512], F32, tag="ps1")
                for h0, hcs in _halves(cs):
                    a, b = c0 + h0, c0 + h0 + hcs
                    for hkp in range(0, HK, 2):  # plain: x_hi @ w1_hi
                        nc.tensor.matmul(
                            ps[:, h0 : h0 + hcs],
                            wt1[:, it, 0, hkp : hkp + 2, :],
                            xt[:, 1, hkp : hkp + 2, a:b],
                            start=(h0 == 0 and hkp == 0),
                            stop=False,
                            perf_mode=DR,
                        )
                return ps

            def s1_rest(it, c0, cs, ps):
                for h0, hcs in _halves(cs):
                    a, b = c0 + h0, c0 + h0 + hcs
                    for hk in range(HK):  # paired: w_hi*x_lo + w_lo*x_hi
                        nc.tensor.matmul(
                            ps[:, h0 : h0 + hcs],
                            wt1[:, it, :, hk, :],
                            xt[:, :, hk, a:b],
                            start=False,
                            stop=(h0 + hcs == cs and hk == HK - 1),
                            perf_mode=DR,
                        )
                # evacuate: hv = psum * sigmoid(psum/SW1) = SW1*silu(z),
                # then split h into e4m3 hi/lo for stage 2
                sg = hvpool.tile([P, 512], F32, tag="sg")
                nc.scalar.activation(
                    sg[:, :cs], ps[:, :cs], AF.Sigmoid, scale=1.0 / SW1
                )
                hv = hvpool.tile([P, 512], F32, tag="hv")
                nc.vector.tensor_mul(out=hv[:, :cs], in0=ps[:, :cs], in1=sg[:, :cs])
                nc.scalar.activation(hlh[:, 1, it, c0 : c0 + cs], hv[:, :cs], AF.Copy)
                nc.gpsimd.tensor_sub(
                    hlh[:, 0, it, c0 : c0 + cs],
                    hv[:, :cs],
                    hlh[:, 1, it, c0 : c0 + cs],
                )

            def s2_group(ht, c0, cs):
                ps = ps2p.tile([P, 512], F32, tag="ps2")
                for h0, hcs in _halves(cs):
                    a, b = c0 + h0, c0 + h0 + hcs
                    for itp in range(0, IT - 1, 2):  # plain: h_hi @ w2_hi
                        nc.tensor.matmul(
                            ps[:, h0 : h0 + hcs],
                            wt2[:, ht, itp : itp + 2, 0, :],
                            hlh[:, 1, itp : itp + 2, a:b],
                            start=(h0 == 0 and itp == 0),
                            stop=False,
                            perf_mode=DR,
                        )
                    # it10 plain, zero-padded second slot
                    nc.tensor.matmul(
                        ps[:, h0 : h0 + hcs],
                        wt2[:, ht, IT - 1, :, :],
                        hlh[:, 1:3, IT - 1, a:b],
                        start=False,
                        stop=False,
                        perf_mode=DR,
                    )
                    for it in range(IT):  # paired: w2_hi*h_lo + w2_lo*h_hi
                        nc.tensor.matmul(
                            ps[:, h0 : h0 + hcs],
                            wt2[:, ht, it, :, :],
                            hlh[:, 0:2, it, a:b],
                            start=False,
                            stop=(h0 + hcs == cs and it == IT - 1),
                            perf_mode=DR,
                        )
                ys = ypool.tile([P, 512], mybir.dt.bfloat16, tag="ys")
                nc.vector.tensor_mul(
                    out=ys[:, :cs], in0=ps[:, :cs], in1=gb[:, c0 : c0 + cs]
                )
                nc.sync.dma_start(y_v[ht][:, c0 : c0 + cs], ys[:, :cs])

            cl = list(zip(c_starts, cks))
            # chunk 0: staggered — run DEPTH groups' plains ahead so the PE
            # has hi-slot work while the lo slots / later w1 slices stream in
            DEPTH = 4
            c0_, cs_ = cl[0]
            pss = {}
            for it in range(min(DEPTH, IT)):
                pss[it] = s1_plains(it, c0_, cs_)
            for it in range(IT):
                s1_rest(it, c0_, cs_, pss.pop(it))
                if it + DEPTH < IT:
                    pss[it + DEPTH] = s1_plains(it + DEPTH, c0_, cs_)
            # software pipeline: stage-2 of chunk i runs after stage-1 of
            # chunk i+1, so the PE stays fed while evac chains drain and the
            # small tail chunk lands last (short epilogue)
            for ci in range(1, len(cl)):
                c0_, cs_ = cl[ci]
                for it in range(IT):
                    s1_rest(it, c0_, cs_, s1_plains(it, c0_, cs_))
                pc0, pcs = cl[ci - 1]
                for ht in range(HK):
                    s2_group(ht, pc0, pcs)
            lc0, lcs = cl[-1]
            for ht in range(HK):
                s2_group(ht, lc0, lcs)

    nc.compile()
    global LAST_NC
    LAST_NC = nc
    return nc


def route(router_logits):
    """Host-side router: softmax -> top-2 -> renormalize."""
    logits = np.asarray(router_logits, dtype=np.float32)
    m = logits.max(axis=-1, keepdims=True)
    ex = np.exp(logits - m)
    probs = ex / ex.sum(axis=-1, keepdims=True)
    order = np.argsort(-probs, axis=-1, kind="stable")[:, :TOPK]
    rows = np.arange(logits.shape[0])[:, None]
    topk_p = probs[rows, order]
    topk_p = topk_p / topk_p.sum(axis=-1, keepdims=True)
    return order, topk_p.astype(np.float32)


def _q8(a):
    return np.asarray(a, dtype=np.float32).astype(E4)


def kernel(x, router_logits, w1, w2):
    x = np.ascontiguousarray(np.asarray(x, dtype=np.float32))
    w1 = np.asarray(w1, dtype=np.float32)
    w2 = np.asarray(w2, dtype=np.float32)
    t = x.shape[0]

    top2_idx, top2_gate = route(router_logits)

    expert_tokens = []
    expert_gates = []
    for e in range(E):
        sel = np.nonzero(top2_idx == e)
        expert_tokens.append(sel[0])
        expert_gates.append(top2_gate[sel[0], sel[1]])
    counts = [len(ix) for ix in expert_tokens]
    count = max(2, max(counts) + max(counts) % 2)

    nc = build_moe_expert_kernel(count)

    in_maps = []
    for e in range(E):
        cnt = counts[e]
        xe = x[expert_tokens[e]]  # [cnt, H]
        x_hi = _q8(xe)
        x_lo = _q8(xe - x_hi.astype(np.float32))
        xlh = np.zeros((P, 2, HK, count), dtype=E4)
        xlh[:, 0, :, :cnt] = x_lo.reshape(cnt, HK, P).transpose(2, 1, 0)
        xlh[:, 1, :, :cnt] = x_hi.reshape(cnt, HK, P).transpose(2, 1, 0)

        W1 = SW1 * w1[e]  # [I, H]
        W1_hi = _q8(W1)
        W1_lo = _q8(W1 - W1_hi.astype(np.float32))
        # whl1[p, it, slot, hk, j] = W1_s[it*128+j, hk*128+p]
        w1hi_t = W1_hi.reshape(IT, P, HK, P).transpose(3, 0, 2, 1)
        w1lo_t = W1_lo.reshape(IT, P, HK, P).transpose(3, 0, 2, 1)
        whl1 = np.stack([w1hi_t, w1lo_t], axis=2)  # [p, it, 2, hk, j]

        W2 = SW2 * w2[e]  # [H, I]
        W2_hi = _q8(W2)
        W2_lo = _q8(W2 - W2_hi.astype(np.float32))
        # whl2[p, ht, it, slot, j] = W2_s[ht*128+j, it*128+p]
        w2hi_t = W2_hi.reshape(HK, P, IT, P).transpose(3, 0, 2, 1)
        w2lo_t = W2_lo.reshape(HK, P, IT, P).transpose(3, 0, 2, 1)
        whl2 = np.stack([w2hi_t, w2lo_t], axis=3)  # [p, ht, it, 2, j]

        g = np.zeros((1, count), dtype=np.float32)
        g[0, :cnt] = expert_gates[e] / (SW1 * SW2)

        in_maps.append(
            {
                "xlh": np.ascontiguousarray(xlh).reshape(P, -1),
                "whl1": np.ascontiguousarray(whl1).reshape(P, -1),
                "whl2": np.ascontiguousarray(whl2).reshape(P, -1),
                "gates": g,
            }
        )

    res = run_bass_kernel_spmd(nc, in_maps, core_ids=list(range(N_CORES)))
    if not all(np.isfinite(r["yT"]).all() for r in res.results):
        # one retry in case of a transient device fault
        res = run_bass_kernel_spmd(nc, in_maps, core_ids=list(range(N_CORES)))

    out = np.zeros((t, H), dtype=np.float32)
    for e in range(E):
        cnt = counts[e]
        out[expert_tokens[e]] += res.results[e]["yT"][:, :cnt].T.astype(np.float32)
    return out
